# revision 1
# baseline (speedup 1.0000x reference)
"""Chamfer L2 distance kernel for 8 Trainium2 NeuronCores — banded KNN version.

Strategy (data-parallel over batch, 2 batches/core, 2 directions/batch):
  Both point clouds are sorted by their z coordinate on the host. Row-tile i
  (128 consecutive sorted X points) only computes distances against a W=512
  window of sorted Y centered at the matching rank — a banded slice of the
  full 4096x4096 distance matrix (8x less work). A per-row certificate
  (banded_rowmin <= gap^2, where gap is the z distance to the nearest
  excluded point) proves the banded min is the true min; the handful of
  uncertified rows (~0.3%) are recomputed exactly on the host.

  The banded H[n, m] = -2<x_n, y_m> + |y_m|^2 is computed via K=21 bf16
  matmuls (exact hi/mid/lo bf16 decomposition, ~1e-7 error) with weights
  rotated across 4 PE row-groups so LDWEIGHTS overlaps matmuls. Row-mins
  come from the fused custom DVE min/min-reduce on PSUM tile halves, with
  ScalarE copying the partner half to SBUF (DVE allows only one PSUM
  operand). The host adds |x_n|^2, certifies, patches, and averages.

Self-contained: hardcodes B=16, N=M=4096, C=3, 8 cores.
"""

import numpy as np
import ml_dtypes

BF = ml_dtypes.bfloat16
B, N, M, C = 16, 4096, 4096, 3
NCORES = 8
BPC = B // NCORES          # batches per core
NU = BPC * 2               # (batch, direction) units per core
K = 21                     # contraction rows (18 product terms + 3 norm rows)
NT = N // 128              # n-tiles per unit
W = 320                    # band width (columns per n-tile), <= SLOT
SLOT = 512                 # PSUM columns reserved per tile (one bank)
TPS = 4                    # tiles per PSUM strip (strip = [128, TPS*SLOT] = 4 banks)
NGROUPS = 3                # PE row-group rotation (base partition 96 unsupported)
CERT_SLACK = 2e-5          # device numerics margin for the certificate
USE_SEGMIN = True          # one segmented DVE op per strip vs per-tile pair ops

_CACHE = {}


def _window_lo_w(i, w):
    c = 128 * i + 64
    return min(max(0, c - w // 2), M - w)


def _window_lo(i):
    """Static window start for tile i (sorted-rank space)."""
    return _window_lo_w(i, W)


# ---------------------------------------------------------------- host prep --

def _split3(v):
    """Exact-ish 3-way bf16 decomposition: h + m + l = v + O(2^-27 |v|)."""
    h = v.astype(BF)
    r = v - h.astype(np.float64)
    m = r.astype(BF)
    r2 = r - m.astype(np.float64)
    l = r2.astype(BF)
    return h, m, l


def _build_tabs(X, Y):
    """X: (N,3) partition side, Y: (M,3) free side.
    Returns lhsT (21, N) bf16 and rhs (21, M) bf16 such that
    (lhsT.T @ rhs)[n, m] ~= -2<X_n, Y_m> + |Y_m|^2 to ~1e-7 absolute."""
    lt = np.empty((K, X.shape[0]), BF)
    rt = np.empty((K, Y.shape[0]), BF)
    Xd = X.astype(np.float64)
    Yd = -2.0 * Y.astype(np.float64)
    row = 0
    for c in range(C):
        Xh, Xm, Xl = _split3(Xd[:, c])
        Yh, Ym, Yl = _split3(Yd[:, c])
        for a, b in ((Xh, Yh), (Xh, Ym), (Xm, Yh), (Xm, Ym), (Xh, Yl), (Xl, Yh)):
            lt[row] = a
            rt[row] = b
            row += 1
    q = np.sum(Y.astype(np.float64) ** 2, axis=1)
    qh, qm, ql = _split3(q)
    ones = np.ones(X.shape[0], BF)
    for qq in (qh, qm, ql):
        lt[row] = ones
        rt[row] = qq
        row += 1
    assert row == K
    return lt, rt


def _unit_xy(pred, target, b, o):
    X = pred[b] if o == 0 else target[b]
    Y = target[b] if o == 0 else pred[b]
    return X, Y


def _sort_perm(P):
    return np.argsort(P[:, 2], kind="stable")


# ------------------------------------------------------------- device build --

def _get_min_min_op():
    if "op" in _CACHE:
        return _CACHE["op"]
    import concourse.dve_ops as dve_ops_mod
    from concourse.dve_ops import DveOp
    from concourse.dve_spec import Spec, Src0, Src1, C0, minn, lower, _has_src1
    from concourse.dve_uop import DveOpSpec

    name = "CHAMFER_MIN_MIN_ANT"
    for op in dve_ops_mod.OPS:
        if op.name == name:
            _CACHE["op"] = op
            return op
    spec = Spec(
        body=minn(Src0, Src1),
        accum=minn,
        accum_init=C0,
        reference=lambda in0, in1, s0, s1, imm2: (
            (b := np.minimum(in0.astype(np.float32), in1.astype(np.float32))),
            np.minimum(
                b.reshape(b.shape[0], -1).min(axis=-1, keepdims=True),
                np.asarray(s0, np.float32).reshape(-1, 1),
            ),
        ),
    )
    if name not in dve_ops_mod._SUB_OPCODE_FOR_NAME:
        row = max(dve_ops_mod._SUB_OPCODE_FOR_NAME.values()) + 1
        assert row < 0x20
        dve_ops_mod._SUB_OPCODE_FOR_NAME[name] = row
    shas = {}
    for ver in ("v3", "v4"):
        try:
            s = DveOpSpec(
                name=name,
                opcode=dve_ops_mod.get_dve_sub_opcode(name),
                uops=lower(spec, ver=ver),
                rd1_en=_has_src1(spec),
            )
            shas[ver] = s.sha(ver)
        except Exception:
            pass
    op = DveOp(name, spec, False, shas)
    dve_ops_mod.OPS.append(op)
    dve_ops_mod.CUSTOM_DVE_SPECS[name] = spec
    _CACHE["op"] = op
    return op


def _get_segmin_op():
    """Segmented row-min op: in0/in1 are [128, S, H] (S segments of H
    columns); body = running min (reset at each segment boundary) of
    min(src0, src1). The destination AP repeats each segment slot H times
    (inner stride 0), so the last write per segment — the segment's min —
    is what lands: out[:, s] = min over the segment. No accumulator read."""
    if "segop" in _CACHE:
        return _CACHE["segop"]
    import dataclasses
    import concourse.dve_ops as dve_ops_mod
    from concourse.dve_ops import DveOp
    import concourse.dve_spec as dve_spec
    from concourse.dve_spec import (
        Spec, Src0, Src1, C0, minn, lower, _has_src1, Scan, AluOp,
    )
    from concourse.dve_uop import DveOpSpec

    name = "CHAMFER_SEGMIN_ANT"
    for op in dve_ops_mod.OPS:
        if op.name == name:
            _CACHE["segop"] = op
            return op

    @dataclasses.dataclass(frozen=True)
    class ResetScan(Scan):
        """Scan that re-seeds from `init` at each SUB_DIM_DONE."""
        _reset_at_subdim = True  # class marker, not a dataclass field

    if not getattr(dve_spec, "_chamfer_reset_patch", False):
        _orig_scan_overrides = dve_spec._scan_overrides

        def _patched_scan_overrides(scans, node_stage):
            seed, step = _orig_scan_overrides(scans, node_stage)
            for sc in scans:
                if getattr(sc, "_reset_at_subdim", False):
                    d = node_stage[sc]
                    step[d] = dve_spec._Stage(
                        sc.op, dve_spec._scan_init(sc), sc.expr)
            return seed, step

        dve_spec._scan_overrides = _patched_scan_overrides
        dve_spec._chamfer_reset_patch = True

    def ref(in0, in1, s0, s1, imm2):
        a = np.minimum(np.asarray(in0, np.float32), np.asarray(in1, np.float32))
        if a.ndim == 2:
            a = a[:, None, :]
        seg = a.min(axis=-1, keepdims=True)
        seg = np.minimum(seg, np.asarray(s0, np.float32).reshape(-1, 1, 1))
        # broadcast so the final memory state matches regardless of the
        # simulator's write order through the stride-0 destination
        return np.broadcast_to(seg, a.shape).copy().reshape(np.shape(in0))

    spec = Spec(
        body=ResetScan(AluOp.MIN, minn(Src0, Src1), init=C0),
        reference=ref,
    )
    if name not in dve_ops_mod._SUB_OPCODE_FOR_NAME:
        row = max(dve_ops_mod._SUB_OPCODE_FOR_NAME.values()) + 1
        assert row < 0x20
        dve_ops_mod._SUB_OPCODE_FOR_NAME[name] = row
    shas = {}
    for ver in ("v3", "v4"):
        try:
            s = DveOpSpec(
                name=name,
                opcode=dve_ops_mod.get_dve_sub_opcode(name),
                uops=lower(spec, ver=ver),
                rd1_en=_has_src1(spec),
            )
            shas[ver] = s.sha(ver)
        except Exception:
            pass
    op = DveOp(name, spec, True, shas)   # subdim=True
    dve_ops_mod.OPS.append(op)
    dve_ops_mod.CUSTOM_DVE_SPECS[name] = spec
    _CACHE["segop"] = op
    return op


def _build_nc(reps=1):
    key = ("nc", reps)
    if key in _CACHE:
        return _CACHE[key]
    import concourse.bacc as bacc
    import concourse.mybir as mybir
    from concourse.tile import TileContext

    MIN_MIN = _get_min_min_op()
    SEGMIN = _get_segmin_op() if USE_SEGMIN else None
    f32 = mybir.dt.float32
    bf16 = mybir.dt.bfloat16
    H = W // 2                 # half-window for the DVE pair trick
    NS = NT // TPS             # strips per unit

    nc = bacc.Bacc(None)
    ltab = nc.dram_tensor("ltab", [NU, K, N], bf16, kind="ExternalInput")
    rtab = nc.dram_tensor("rtab", [NU, K, M], bf16, kind="ExternalInput")
    outt = nc.dram_tensor("out", [128, NU * NT], f32, kind="ExternalOutput")

    with TileContext(nc) as tc:
        with (
            tc.tile_pool(name="stage", bufs=2) as stage,
            tc.tile_pool(name="psum", bufs=2, space="PSUM") as psum,
            tc.tile_pool(name="cpp", bufs=3) as cpp,
            tc.tile_pool(name="res", bufs=1) as res,
        ):
            raw = res.tile([128, NU * NT], f32, tag="raw")
            dummies = [res.tile([128, 1], f32, tag=f"dummy{d}", name=f"dummy{d}")
                       for d in range(4)]
            for _rep in range(reps):
              for u in range(NU):
                lt = stage.tile([128, N], bf16, tag="lt", name="lt")
                rt = stage.tile([128, M], bf16, tag="rt", name="rt")
                for g in range(NGROUPS):
                    nc.sync.dma_start(out=lt[32 * g:32 * g + K, :], in_=ltab[u])
                    nc.sync.dma_start(out=rt[32 * g:32 * g + K, :], in_=rtab[u])
                for s in range(NS):
                    strip = psum.tile([128, TPS * SLOT], f32, tag="strip", name="strip")
                    for j in range(TPS):
                        i = s * TPS + j
                        g = 32 * (i % NGROUPS)
                        lo = _window_lo(i)
                        nc.tensor.matmul(
                            strip[:, SLOT * j:SLOT * j + W],
                            lt[g:g + K, 128 * i:128 * (i + 1)],
                            rt[g:g + K, lo:lo + W],
                            start=True, stop=True)
                    cp = cpp.tile([128, TPS * H], f32, tag="cp", name="cp")
                    if USE_SEGMIN:
                        strip3 = strip[:, :].rearrange("p (s w) -> p s w", w=SLOT)
                        cp3 = cp[:, :].rearrange("p (s h) -> p s h", h=H)
                        nc.scalar.copy(out=cp3, in_=strip3[:, :, H:2 * H])
                        slot0 = u * NT + s * TPS
                        nc.vector._custom_dve(
                            SEGMIN,
                            out=raw[:, slot0:slot0 + TPS]
                                .unsqueeze(-1).broadcast_to((128, TPS, H)),
                            in0=strip3[:, :, 0:H],
                            in1=cp3,
                            s0=1.0e30,
                        )
                    else:
                        for j in range(TPS):
                            nc.scalar.copy(out=cp[:, H * j:H * (j + 1)],
                                           in_=strip[:, SLOT * j + H:SLOT * j + 2 * H])
                        for j in range(TPS):
                            i = s * TPS + j
                            slot = u * NT + i
                            nc.vector._custom_dve(
                                MIN_MIN,
                                out=dummies[slot % 4].broadcast_to(cp[:, :H].shape),
                                in0=strip[:, SLOT * j:SLOT * j + H],
                                in1=cp[:, H * j:H * (j + 1)],
                                s0=1.0e30,
                                accum_out=raw[:, slot:slot + 1],
                            )
            nc.sync.dma_start(out=outt[:, :], in_=raw[:, :])
    nc.compile()
    _CACHE[key] = nc
    return nc


# -------------------------------------------------------------------- entry --

def _prepare_inputs(pred, target):
    ltabs = np.empty((NCORES, NU, K, N), BF)
    rtabs = np.empty((NCORES, NU, K, M), BF)
    for core in range(NCORES):
        for lb in range(BPC):
            b = core * BPC + lb
            for o in range(2):
                X, Y = _unit_xy(pred, target, b, o)
                Xs = X[_sort_perm(X)]
                Ys = Y[_sort_perm(Y)]
                lt, rt = _build_tabs(Xs, Ys)
                u = lb * 2 + o
                ltabs[core, u] = lt
                rtabs[core, u] = rt
    return ltabs, rtabs


def _postprocess(results, pred, target):
    losses = []
    n_fallback = 0
    for core in range(NCORES):
        out = np.asarray(results[core]["out"])  # (128, NU*NT)
        for lb in range(BPC):
            b = core * BPC + lb
            total = 0.0
            for o in range(2):
                u = lb * 2 + o
                X, Y = _unit_xy(pred, target, b, o)
                px = _sort_perm(X)
                py = _sort_perm(Y)
                Xs = X[px].astype(np.float64)
                Ys = Y[py].astype(np.float64)
                kx = Xs[:, 2]
                ky = Ys[:, 2]

                sl = out[:, u * NT:(u + 1) * NT]          # (128, NT), [p, i]
                hmin = sl.T.reshape(-1).astype(np.float64)  # n = 128*i + p
                # |x~|^2 from the exact bf16 splits used on device
                xt = np.zeros_like(Xs)
                for c in range(C):
                    h, m, l = _split3(Xs[:, c])
                    xt[:, c] = (h.astype(np.float64) + m.astype(np.float64)
                                + l.astype(np.float64))
                rowmin = hmin + np.sum(xt * xt, axis=1)

                # certificate: distance to nearest excluded z
                g = np.full(N, np.inf)
                for i in range(NT):
                    rows = slice(128 * i, 128 * i + 128)
                    lo = _window_lo(i)
                    glo = kx[rows] - ky[lo] if lo > 0 else np.inf
                    ghi = ky[lo + W - 1] - kx[rows] if lo + W < M else np.inf
                    g[rows] = np.minimum(glo, ghi)
                bad = rowmin > g * g - CERT_SLACK
                if bad.any():
                    n_fallback += int(bad.sum())
                    d = ((Xs[bad, None, :] - Ys[None, :, :]) ** 2).sum(-1)
                    rowmin[bad] = d.min(axis=1)
                total += rowmin.mean()
            losses.append(total)
    _CACHE["n_fallback"] = n_fallback
    return np.float32(np.mean(losses))


def _run(pred, target, trace=False):
    from concourse.bass_utils import run_bass_kernel_spmd

    pred = np.asarray(pred, dtype=np.float32)
    target = np.asarray(target, dtype=np.float32)
    assert pred.shape == (B, N, C) and target.shape == (B, M, C)
    ltabs, rtabs = _prepare_inputs(pred, target)
    nc = _build_nc()
    in_maps = [{"ltab": ltabs[c], "rtab": rtabs[c]} for c in range(NCORES)]
    try:
        res = run_bass_kernel_spmd(nc, in_maps, core_ids=list(range(NCORES)),
                                   trace=trace)
    except Exception:
        res = run_bass_kernel_spmd(nc, in_maps, core_ids=list(range(NCORES)),
                                   trace=trace)
    return _postprocess(res.results, pred, target), res


def kernel(pred, target):
    loss, _ = _run(pred, target, trace=False)
    return loss



# revision 13
# speedup vs baseline: 1.7253x; 1.7253x over previous
"""Chamfer L2 distance kernel for 8 Trainium2 NeuronCores — fused banded KNN.

Strategy (data-parallel over batch, 2 batches/core, both chamfer directions
extracted from ONE banded distance computation per batch):
  Both point clouds are sorted by their z coordinate on the host. Row-tile i
  (128 consecutive sorted X points) computes -d^2 against a W=192 window of
  sorted Y centered at the matching rank, via K=24 bf16 matmuls (exact
  hi/mid/lo bf16 decomposition of the products AND both squared norms, so
  PSUM holds the full negated squared distance, ~1e-5 accurate).  Weights
  rotate across 3 PE row-groups so LDWEIGHTS overlaps matmuls.

  X-side row-mins (min over the window) come from the fused custom DVE
  seg-max on PSUM tile halves (PSUM holds -d^2; max of -d^2 = -min d^2),
  with ScalarE copying the partner half to SBUF (DVE allows only one PSUM
  operand).  Y-side column-mins come from the Pool engine's native
  partition_all_reduce(max) over the central Wp=176 columns of each tile —
  the same PSUM is reduced along both axes, so each batch needs only ONE
  matmul pass instead of two.

  Host-side: per-row/per-column z-gap certificates (banded min <= gap^2
  proves the banded min is the true min); uncertified rows/columns are
  recomputed exactly on the host.  The host also averages into the scalar
  loss.

Self-contained: hardcodes B=16, N=M=4096, C=3, 8 cores.
"""

import numpy as np
import ml_dtypes

BF = ml_dtypes.bfloat16
B, N, M, C = 16, 4096, 4096, 3
NCORES = 8
BPC = B // NCORES          # batches per core
K = 24                     # contraction rows (18 product + 3 |y|^2 + 3 |x|^2)
NT = N // 128              # n-tiles per batch
W = 192                    # band width (columns per n-tile), <= SLOT
WP = 160                   # Pool column-reduce slice width per tile
WPAD = WP + 8              # padded per-tile stride in the colmin scratch (keeps
                           # the Pool out-AP 3D/non-mergeable)
OFF = (W - WP) // 2        # slice offset inside the window
CD = 32                    # columns per tile copied PSUM->SBUF by DVE (rest: ACT)
SLOT = 512                 # PSUM columns per tile (matmul outs must be
                           # PSUM-bank aligned: 512 f32 = one 2KB bank)
TPS = 4                    # tiles per PSUM strip (strip = [128, TPS*SLOT] = 4 banks)
NS = NT // TPS             # strips per batch
NGROUPS = 3                # PE row-group rotation (base partition 96 unsupported)
GT = (NT + NGROUPS - 1) // NGROUPS   # max tiles per group (11)
CERT_SLACK = 5e-5          # device numerics margin for the certificate

_CACHE = {}
_EN_POOL = True      # debug: emit the Pool column-reduce
_EN_SEGMAX = True    # debug: emit the DVE segmented row-max


def _window_lo(i):
    """Static window start for tile i (sorted-rank space)."""
    c = 128 * i + 64
    return min(max(0, c - W // 2), M - W)


# ---------------------------------------------------------------- host prep --

def _split3(v):
    """Exact-ish 3-way bf16 decomposition: h + m + l = v + O(2^-27 |v|)."""
    h = v.astype(BF)
    r = v - h.astype(np.float64)
    m = r.astype(BF)
    r2 = r - m.astype(np.float64)
    l = r2.astype(BF)
    return h, m, l


def _build_tabs(X, Y):
    """X: (N,3) partition side, Y: (M,3) free side.
    Returns lt (K, N) bf16 and rt (K, M) bf16 such that
    (lt.T @ rt)[n, m] ~= -(|X_n - Y_m|^2) to ~1e-5 absolute."""
    lt = np.empty((K, X.shape[0]), BF)
    rt = np.empty((K, Y.shape[0]), BF)
    Xd = X.astype(np.float64)
    Yd = -2.0 * Y.astype(np.float64)
    row = 0
    for c in range(C):
        Xh, Xm, Xl = _split3(Xd[:, c])
        Yh, Ym, Yl = _split3(Yd[:, c])
        for a, b in ((Xh, Yh), (Xh, Ym), (Xm, Yh), (Xm, Ym), (Xh, Yl), (Xl, Yh)):
            lt[row] = a
            rt[row] = b
            row += 1
    ones_x = np.ones(X.shape[0], BF)
    ones_y = np.ones(Y.shape[0], BF)
    q = np.sum(Y.astype(np.float64) ** 2, axis=1)
    for qq in _split3(q):
        lt[row] = ones_x
        rt[row] = qq
        row += 1
    r = np.sum(X.astype(np.float64) ** 2, axis=1)
    for rr in _split3(r):
        lt[row] = rr
        rt[row] = ones_y
        row += 1
    assert row == K
    # negate so PSUM = -d^2 (Pool partition reduce only supports max)
    return -lt, rt


def _sort_perm(P):
    return np.argsort(P[:, 2], kind="stable")


# ------------------------------------------------------------- device build --

def _get_segmax_op():
    """Segmented row-max op: in0/in1 are [128, S, H] (S segments of H
    columns); body = running max (reset at each segment boundary) of
    max(src0, src1). The destination AP repeats each segment slot H times
    (inner stride 0), so the last write per segment — the segment's max —
    is what lands: out[:, s] = max over the segment. No accumulator read."""
    if "segop" in _CACHE:
        return _CACHE["segop"]
    import dataclasses
    import concourse.dve_ops as dve_ops_mod
    from concourse.dve_ops import DveOp
    import concourse.dve_spec as dve_spec
    from concourse.dve_spec import (
        Spec, Src0, Src1, C0, maxx, lower, _has_src1, Scan, AluOp,
    )
    from concourse.dve_uop import DveOpSpec

    name = "CHAMFER_SEGMAX_ANT"
    for op in dve_ops_mod.OPS:
        if op.name == name:
            _CACHE["segop"] = op
            return op

    @dataclasses.dataclass(frozen=True)
    class ResetScan(Scan):
        """Scan that re-seeds from `init` at each SUB_DIM_DONE."""
        _reset_at_subdim = True  # class marker, not a dataclass field

    if not getattr(dve_spec, "_chamfer_reset_patch", False):
        _orig_scan_overrides = dve_spec._scan_overrides

        def _patched_scan_overrides(scans, node_stage):
            seed, step = _orig_scan_overrides(scans, node_stage)
            for sc in scans:
                if getattr(sc, "_reset_at_subdim", False):
                    d = node_stage[sc]
                    step[d] = dve_spec._Stage(
                        sc.op, dve_spec._scan_init(sc), sc.expr)
            return seed, step

        dve_spec._scan_overrides = _patched_scan_overrides
        dve_spec._chamfer_reset_patch = True

    def ref(in0, in1, s0, s1, imm2):
        a = np.maximum(np.asarray(in0, np.float32), np.asarray(in1, np.float32))
        if a.ndim == 2:
            a = a[:, None, :]
        seg = a.max(axis=-1, keepdims=True)
        seg = np.maximum(seg, np.asarray(s0, np.float32).reshape(-1, 1, 1))
        # broadcast so the final memory state matches regardless of the
        # simulator's write order through the stride-0 destination
        return np.broadcast_to(seg, a.shape).copy().reshape(np.shape(in0))

    spec = Spec(
        body=ResetScan(AluOp.MAX, maxx(Src0, Src1), init=C0),
        reference=ref,
    )
    if name not in dve_ops_mod._SUB_OPCODE_FOR_NAME:
        row = max(dve_ops_mod._SUB_OPCODE_FOR_NAME.values()) + 1
        assert row < 0x20
        dve_ops_mod._SUB_OPCODE_FOR_NAME[name] = row
    shas = {}
    for ver in ("v3", "v4"):
        try:
            s = DveOpSpec(
                name=name,
                opcode=dve_ops_mod.get_dve_sub_opcode(name),
                uops=lower(spec, ver=ver),
                rd1_en=_has_src1(spec),
            )
            shas[ver] = s.sha(ver)
        except Exception:
            pass
    op = DveOp(name, spec, True, shas)   # subdim=True
    dve_ops_mod.OPS.append(op)
    dve_ops_mod.CUSTOM_DVE_SPECS[name] = spec
    _CACHE["segop"] = op
    return op


def _build_nc(reps=1):
    key = ("nc", reps)
    if key in _CACHE:
        return _CACHE[key]
    import concourse.bacc as bacc
    import concourse.bass_isa as bass_isa
    import concourse.mybir as mybir
    from concourse.tile import TileContext

    SEGMAX = _get_segmax_op()
    f32 = mybir.dt.float32
    bf16 = mybir.dt.bfloat16
    H = W // 2                 # half-window for the DVE pair trick

    nc = bacc.Bacc(None)
    ltab = nc.dram_tensor("ltab", [BPC, NGROUPS, K, GT * 128], bf16,
                          kind="ExternalInput")
    rtab = nc.dram_tensor("rtab", [BPC, NGROUPS, K, GT * W], bf16,
                          kind="ExternalInput")
    outt = nc.dram_tensor("out", [128, BPC * NT], f32, kind="ExternalOutput")
    colo = nc.dram_tensor("colo", [1, BPC * NS * TPS * WP], f32,
                          kind="ExternalOutput")
    NTILE_TOT = BPC * NS * TPS

    with TileContext(nc) as tc:
        with (
            tc.tile_pool(name="stage", bufs=2) as stage,
            tc.tile_pool(name="psum", bufs=2, space="PSUM") as psum,
            tc.tile_pool(name="cpp", bufs=3) as cpp,
            tc.tile_pool(name="res", bufs=1) as res,
        ):
            raw = res.tile([128, BPC * NT], f32, tag="raw")
            colp = res.tile([128, NTILE_TOT * WPAD], f32, tag="colp")
            nc.vector.memset(raw[:, :], 0.0)
            nc.vector.memset(colp[:, :], 0.0)
            for _rep in range(reps):
              for u in range(BPC):
                lt = stage.tile([128, GT * 128], bf16, tag="lt", name="lt")
                rt = stage.tile([128, GT * W], bf16, tag="rt", name="rt")
                for g in range(NGROUPS):
                    nc.sync.dma_start(out=lt[32 * g:32 * g + K, :],
                                      in_=ltab[u, g])
                    nc.sync.dma_start(out=rt[32 * g:32 * g + K, :],
                                      in_=rtab[u, g])
                for s in range(NS):
                    strip = psum.tile([128, TPS * SLOT], f32, tag="strip",
                                      name="strip")
                    for j in range(TPS):
                        i = s * TPS + j
                        g, tg = i % NGROUPS, i // NGROUPS
                        nc.tensor.matmul(
                            strip[:, SLOT * j:SLOT * j + W],
                            lt[32 * g:32 * g + K, 128 * tg:128 * (tg + 1)],
                            rt[32 * g:32 * g + K, W * tg:W * tg + W],
                            start=True, stop=True)
                    strip3 = strip[:, :].rearrange("p (s w) -> p s w", w=SLOT)
                    # Drain PSUM -> SBUF exactly once (GPSIMD cannot read
                    # PSUM): DVE copies the first CD columns, ACT the rest.
                    cp = cpp.tile([128, TPS * W], f32, tag="cp", name="cp")
                    cp3 = cp[:, :].rearrange("p (s w) -> p s w", w=W)
                    nc.vector.tensor_copy(out=cp3[:, :, 0:CD],
                                          in_=strip3[:, :, 0:CD])
                    nc.scalar.copy(out=cp3[:, :, CD:W],
                                   in_=strip3[:, :, CD:W])
                    # X side: fused segmented max over window halves
                    slot0 = u * NT + s * TPS
                    if _EN_SEGMAX:
                        nc.vector._custom_dve(
                            SEGMAX,
                            out=raw[:, slot0:slot0 + TPS]
                                .unsqueeze(-1).broadcast_to((128, TPS, H)),
                            in0=cp3[:, :, 0:H],
                            in1=cp3[:, :, H:2 * H],
                            s0=-1.0e30,
                        )
                    # Y side: partition max over the central WP columns
                    if _EN_POOL:
                        cbase = (u * NS + s) * TPS * WPAD
                        nc.gpsimd.partition_all_reduce(
                            colp[:, cbase:cbase + TPS * WPAD]
                                .rearrange("p (s w) -> p s w", w=WPAD)[:, :, 0:WP],
                            cp3[:, :, OFF:OFF + WP],
                            channels=128,
                            reduce_op=bass_isa.ReduceOp.max,
                        )
            nc.sync.dma_start(out=outt[:, :], in_=raw[:, :])
            nc.sync.dma_start(
                out=colo[:, :].rearrange("p (t w) -> p t w", w=WP),
                in_=colp[0:1, :].rearrange("p (t w) -> p t w", w=WPAD)[:, :, 0:WP])
    nc.compile()
    _CACHE[key] = nc
    return nc


# -------------------------------------------------------------------- entry --

def _prepare_inputs(pred, target):
    ltabs = np.zeros((NCORES, BPC, NGROUPS, K, GT * 128), BF)
    rtabs = np.zeros((NCORES, BPC, NGROUPS, K, GT * W), BF)
    for core in range(NCORES):
        for u in range(BPC):
            b = core * BPC + u
            X, Y = pred[b], target[b]
            Xs = X[_sort_perm(X)]
            Ys = Y[_sort_perm(Y)]
            lt, rt = _build_tabs(Xs, Ys)
            for i in range(NT):
                g, tg = i % NGROUPS, i // NGROUPS
                ltabs[core, u, g, :, 128 * tg:128 * (tg + 1)] = \
                    lt[:, 128 * i:128 * (i + 1)]
                lo = _window_lo(i)
                rtabs[core, u, g, :, W * tg:W * tg + W] = rt[:, lo:lo + W]
    return ltabs, rtabs


def _postprocess(results, pred, target):
    losses = []
    n_fb_row = 0
    n_fb_col = 0
    for core in range(NCORES):
        raw = np.asarray(results[core]["out"])    # (128, BPC*NT)
        colo = np.asarray(results[core]["colo"]).reshape(-1)
        for u in range(BPC):
            b = core * BPC + u
            X, Y = pred[b], target[b]
            Xs = X[_sort_perm(X)].astype(np.float64)
            Ys = Y[_sort_perm(Y)].astype(np.float64)
            kx = Xs[:, 2]
            ky = Ys[:, 2]

            # ---- X side (row mins) ----
            sl = raw[:, u * NT:(u + 1) * NT]            # (128, NT), [p, i]
            rowmin = -sl.T.reshape(-1).astype(np.float64)   # n = 128*i + p

            g = np.full(N, np.inf)
            for i in range(NT):
                rows = slice(128 * i, 128 * i + 128)
                lo = _window_lo(i)
                glo = np.maximum(kx[rows] - ky[lo - 1], 0) if lo > 0 else np.inf
                ghi = (np.maximum(ky[lo + W] - kx[rows], 0)
                       if lo + W < M else np.inf)
                g[rows] = np.minimum(glo, ghi)
            bad = rowmin > g * g - CERT_SLACK
            if bad.any():
                n_fb_row += int(bad.sum())
                d = ((Xs[bad, None, :] - Ys[None, :, :]) ** 2).sum(-1)
                rowmin[bad] = d.min(axis=1)

            # ---- Y side (column mins) ----
            colmin = np.full(M, np.inf)
            cov_lo = np.full(M, NT, dtype=np.int64)
            cov_hi = np.full(M, -1, dtype=np.int64)
            cbase = u * NS * TPS * WP
            for i in range(NT):
                lo = _window_lo(i)
                s0 = lo + OFF
                vals = -colo[cbase + i * WP: cbase + (i + 1) * WP]
                seg = slice(s0, s0 + WP)
                colmin[seg] = np.minimum(colmin[seg], vals)
                cov_lo[seg] = np.minimum(cov_lo[seg], i)
                cov_hi[seg] = np.maximum(cov_hi[seg], i)
            covered = cov_hi >= cov_lo
            lo_idx = cov_lo * 128 - 1
            gy_lo = np.where(lo_idx >= 0,
                             np.maximum(ky - kx[np.clip(lo_idx, 0, N - 1)], 0),
                             np.inf)
            hi_idx = (cov_hi + 1) * 128
            gy_hi = np.where(hi_idx < N,
                             np.maximum(kx[np.clip(hi_idx, 0, N - 1)] - ky, 0),
                             np.inf)
            gy = np.minimum(gy_lo, gy_hi)
            badc = (~covered) | (colmin > gy * gy - CERT_SLACK)
            if badc.any():
                n_fb_col += int(badc.sum())
                d = ((Xs[None, :, :] - Ys[badc][:, None, :]) ** 2).sum(-1)
                colmin[badc] = d.min(axis=1)

            losses.append(rowmin.mean() + colmin.mean())
    _CACHE["n_fallback"] = (n_fb_row, n_fb_col)
    return np.float32(np.mean(losses))


def _run(pred, target, trace=False):
    from concourse.bass_utils import run_bass_kernel_spmd

    pred = np.asarray(pred, dtype=np.float32)
    target = np.asarray(target, dtype=np.float32)
    assert pred.shape == (B, N, C) and target.shape == (B, M, C)
    ltabs, rtabs = _prepare_inputs(pred, target)
    nc = _build_nc()
    in_maps = [{"ltab": ltabs[c], "rtab": rtabs[c]} for c in range(NCORES)]
    try:
        res = run_bass_kernel_spmd(nc, in_maps, core_ids=list(range(NCORES)),
                                   trace=trace)
    except Exception:
        res = run_bass_kernel_spmd(nc, in_maps, core_ids=list(range(NCORES)),
                                   trace=trace)
    return _postprocess(res.results, pred, target), res


def kernel(pred, target):
    loss, _ = _run(pred, target, trace=False)
    return loss


# revision 22
# speedup vs baseline: 3.1734x; 1.8394x over previous
"""Chamfer L2 distance kernel for 8 Trainium2 NeuronCores — banded KNN.

Strategy (data-parallel over batch, 2 batches/core, 2 directions/batch):
  Both point clouds are sorted by their z coordinate on the host.  For each
  direction (pred->target and target->pred) a banded pass runs on device:
  row-tile i (128 consecutive sorted query points) computes the negated
  squared distance -d^2 against a W=192 window of the sorted other cloud
  centered at the matching rank, via K=24 bf16 matmuls (exact hi/mid/lo bf16
  decomposition of the cross products AND both squared norms, so PSUM holds
  the full -d^2 to ~1e-6).  Weights rotate across 3 PE row-groups so
  LDWEIGHTS overlaps matmuls.  Row-maxes of -d^2 (= negated row-mins of d^2)
  come from the fused custom DVE seg-max over window halves, with ScalarE
  copying the upper half to SBUF (DVE allows only one PSUM operand).

  Optionally (POOL_BATCHES > 0) the target->pred direction of some batches
  is instead extracted from the SAME pass as pred->target: the Pool engine's
  partition_all_reduce(max) reduces the central WP columns of each tile
  across partitions, giving banded column-mins.  Measured on HW the gpsimd
  op serializes badly (~0.8us/op), so the default is POOL_BATCHES=0 (two
  separate passes per batch).

  Host-side: per-row z-gap certificates (banded_min <= gap^2 proves the
  banded min is the true min); uncertified rows are recomputed exactly on
  the host, which also averages into the scalar loss.

Self-contained: hardcodes B=16, N=M=4096, C=3, 8 cores.
"""

import numpy as np
import ml_dtypes

BF = ml_dtypes.bfloat16
B, N, M, C = 16, 4096, 4096, 3
NCORES = 8
BPC = B // NCORES          # batches per core
K = 24                     # contraction rows (18 product + 3 |y|^2 + 3 |x|^2)
NT = N // 128              # n-tiles per pass
W = 192                    # band width (columns per n-tile)
WP = 160                   # Pool column-reduce slice width per tile
WPAD = WP + 8              # padded per-tile stride in the colmin scratch (keeps
                           # the Pool out-AP 3D/non-mergeable)
OFF = (W - WP) // 2        # Pool slice offset inside the window
SLOT = 512                 # PSUM columns per tile (matmul outs must be
                           # PSUM-bank aligned: 512 f32 = one 2KB bank)
TPS = 4                    # tiles per PSUM strip (strip = [128, TPS*SLOT] = 4 banks)
NS = NT // TPS             # strips per pass
NGROUPS = 3                # PE row-group rotation (base partition 96 unsupported)
GT = (NT + NGROUPS - 1) // NGROUPS   # max tiles per group (11)
CERT_SLACK = 5e-5          # device numerics margin for the certificate
POOL_BATCHES = 0           # batches/core whose Y side rides the X pass via Pool

_CACHE = {}
_EN_POOL = True      # debug: emit the Pool column-reduce
_EN_SEGMAX = True    # debug: emit the DVE segmented row-max
_SEGMAX_REPS = None  # debug: segmax multiplier (load probing)


def _units():
    """Per-core device pass list: (local_batch, orient); orient 0 = rows are
    pred (X side), 1 = rows are target (Y side, only for non-Pool batches)."""
    us = []
    for b in range(BPC):
        us.append((b, 0))
        if b >= POOL_BATCHES:
            us.append((b, 1))
    return us


def _window_lo(i):
    """Static window start for tile i (sorted-rank space)."""
    c = 128 * i + 64
    return min(max(0, c - W // 2), M - W)


# ---------------------------------------------------------------- host prep --

def _split3(v):
    """Exact-ish 3-way bf16 decomposition: h + m + l = v + O(2^-27 |v|)."""
    h = v.astype(BF)
    r = v - h.astype(np.float64)
    m = r.astype(BF)
    r2 = r - m.astype(np.float64)
    l = r2.astype(BF)
    return h, m, l


def _build_tabs(X, Y):
    """X: (N,3) partition side, Y: (M,3) free side.
    Returns lt (K, N) bf16 and rt (K, M) bf16 such that
    (lt.T @ rt)[n, m] ~= -(|X_n - Y_m|^2) to ~1e-6 absolute."""
    lt = np.empty((K, X.shape[0]), BF)
    rt = np.empty((K, Y.shape[0]), BF)
    Xd = X.astype(np.float64)
    Yd = -2.0 * Y.astype(np.float64)
    row = 0
    for c in range(C):
        Xh, Xm, Xl = _split3(Xd[:, c])
        Yh, Ym, Yl = _split3(Yd[:, c])
        for a, b in ((Xh, Yh), (Xh, Ym), (Xm, Yh), (Xm, Ym), (Xh, Yl), (Xl, Yh)):
            lt[row] = a
            rt[row] = b
            row += 1
    ones_x = np.ones(X.shape[0], BF)
    ones_y = np.ones(Y.shape[0], BF)
    q = np.sum(Y.astype(np.float64) ** 2, axis=1)
    for qq in _split3(q):
        lt[row] = ones_x
        rt[row] = qq
        row += 1
    r = np.sum(X.astype(np.float64) ** 2, axis=1)
    for rr in _split3(r):
        lt[row] = rr
        rt[row] = ones_y
        row += 1
    assert row == K
    # negate so PSUM = -d^2 (all reductions become max)
    return -lt, rt


def _sort_perm(P):
    return np.argsort(P[:, 2], kind="stable")


# ------------------------------------------------------------- device build --

def _get_segmax_op():
    """Segmented row-max op: in0/in1 are [128, S, H] (S segments of H
    columns); body = running max (reset at each segment boundary) of
    max(src0, src1). The destination AP repeats each segment slot H times
    (inner stride 0), so the last write per segment — the segment's max —
    is what lands: out[:, s] = max over the segment. No accumulator read."""
    if "segop" in _CACHE:
        return _CACHE["segop"]
    import dataclasses
    import concourse.dve_ops as dve_ops_mod
    from concourse.dve_ops import DveOp
    import concourse.dve_spec as dve_spec
    from concourse.dve_spec import (
        Spec, Src0, Src1, C0, maxx, lower, _has_src1, Scan, AluOp,
    )
    from concourse.dve_uop import DveOpSpec

    name = "CHAMFER_SEGMAX_ANT"
    for op in dve_ops_mod.OPS:
        if op.name == name:
            _CACHE["segop"] = op
            return op

    @dataclasses.dataclass(frozen=True)
    class ResetScan(Scan):
        """Scan that re-seeds from `init` at each SUB_DIM_DONE."""
        _reset_at_subdim = True  # class marker, not a dataclass field

    if not getattr(dve_spec, "_chamfer_reset_patch", False):
        _orig_scan_overrides = dve_spec._scan_overrides

        def _patched_scan_overrides(scans, node_stage):
            seed, step = _orig_scan_overrides(scans, node_stage)
            for sc in scans:
                if getattr(sc, "_reset_at_subdim", False):
                    d = node_stage[sc]
                    step[d] = dve_spec._Stage(
                        sc.op, dve_spec._scan_init(sc), sc.expr)
            return seed, step

        dve_spec._scan_overrides = _patched_scan_overrides
        dve_spec._chamfer_reset_patch = True

    def ref(in0, in1, s0, s1, imm2):
        a = np.maximum(np.asarray(in0, np.float32), np.asarray(in1, np.float32))
        if a.ndim == 2:
            a = a[:, None, :]
        seg = a.max(axis=-1, keepdims=True)
        seg = np.maximum(seg, np.asarray(s0, np.float32).reshape(-1, 1, 1))
        # broadcast so the final memory state matches regardless of the
        # simulator's write order through the stride-0 destination
        return np.broadcast_to(seg, a.shape).copy().reshape(np.shape(in0))

    spec = Spec(
        body=ResetScan(AluOp.MAX, maxx(Src0, Src1), init=C0),
        reference=ref,
    )
    if name not in dve_ops_mod._SUB_OPCODE_FOR_NAME:
        row = max(dve_ops_mod._SUB_OPCODE_FOR_NAME.values()) + 1
        assert row < 0x20
        dve_ops_mod._SUB_OPCODE_FOR_NAME[name] = row
    shas = {}
    for ver in ("v3", "v4"):
        try:
            s = DveOpSpec(
                name=name,
                opcode=dve_ops_mod.get_dve_sub_opcode(name),
                uops=lower(spec, ver=ver),
                rd1_en=_has_src1(spec),
            )
            shas[ver] = s.sha(ver)
        except Exception:
            pass
    op = DveOp(name, spec, True, shas)   # subdim=True
    dve_ops_mod.OPS.append(op)
    dve_ops_mod.CUSTOM_DVE_SPECS[name] = spec
    _CACHE["segop"] = op
    return op


def _build_nc(reps=1):
    key = ("nc", reps)
    if key in _CACHE:
        return _CACHE[key]
    import concourse.bacc as bacc
    import concourse.bass_isa as bass_isa
    import concourse.mybir as mybir
    from concourse.tile import TileContext

    SEGMAX = _get_segmax_op()
    f32 = mybir.dt.float32
    bf16 = mybir.dt.bfloat16
    H = W // 2                 # half-window for the DVE pair trick
    units = _units()
    NU = len(units)
    NPOOL = POOL_BATCHES * NS * TPS   # pool-reduced tiles per core

    nc = bacc.Bacc(None)
    ltab = nc.dram_tensor("ltab", [NU, NGROUPS, K, GT * 128], bf16,
                          kind="ExternalInput")
    rtab = nc.dram_tensor("rtab", [NU, NGROUPS, K, GT * W], bf16,
                          kind="ExternalInput")
    outt = nc.dram_tensor("out", [128, NU * NT], f32, kind="ExternalOutput")
    colo = (nc.dram_tensor("colo", [1, NPOOL * WP], f32, kind="ExternalOutput")
            if NPOOL else None)

    with TileContext(nc) as tc:
        with (
            tc.tile_pool(name="stage", bufs=2) as stage,
            tc.tile_pool(name="psum", bufs=2, space="PSUM") as psum,
            tc.tile_pool(name="cpp", bufs=6) as cpp,
            tc.tile_pool(name="res", bufs=1) as res,
        ):
            raw = res.tile([128, NU * NT], f32, tag="raw")
            nc.vector.memset(raw[:, :], 0.0)
            if NPOOL:
                colp = res.tile([128, NPOOL * WPAD], f32, tag="colp")
                nc.vector.memset(colp[:, :], 0.0)
            for _rep in range(reps):
              for ui, (ub, orient) in enumerate(units):
                fused = orient == 0 and ub < POOL_BATCHES
                cplo = min(OFF, H) if fused else H
                lt = stage.tile([128, GT * 128], bf16, tag="lt", name="lt")
                rt = stage.tile([128, GT * W], bf16, tag="rt", name="rt")
                for g in range(NGROUPS):
                    nc.sync.dma_start(out=lt[32 * g:32 * g + K, :],
                                      in_=ltab[ui, g])
                    nc.sync.dma_start(out=rt[32 * g:32 * g + K, :],
                                      in_=rtab[ui, g])
                for s in range(NS):
                    strip = psum.tile([128, TPS * SLOT], f32, tag="strip",
                                      name="strip")
                    for j in range(TPS):
                        i = s * TPS + j
                        g, tg = i % NGROUPS, i // NGROUPS
                        nc.tensor.matmul(
                            strip[:, SLOT * j:SLOT * j + W],
                            lt[32 * g:32 * g + K, 128 * tg:128 * (tg + 1)],
                            rt[32 * g:32 * g + K, W * tg:W * tg + W],
                            start=True, stop=True)
                    strip3 = strip[:, :].rearrange("p (s w) -> p s w", w=SLOT)
                    # ScalarE copies the SBUF-resident part of the window
                    # (upper half for the DVE pair; plus the Pool slice for
                    # fused strips — GPSIMD cannot read PSUM).
                    cp = cpp.tile([128, TPS * W], f32, tag="cp", name="cp")
                    cp3 = cp[:, :].rearrange("p (s w) -> p s w", w=W)
                    nc.scalar.copy(out=cp3[:, :, cplo:W],
                                   in_=strip3[:, :, cplo:W])
                    # row maxes: fused segmented max over window halves
                    slot0 = ui * NT + s * TPS
                    if _EN_SEGMAX:
                        for _r in range(_SEGMAX_REPS or 1):
                            nc.vector._custom_dve(
                                SEGMAX,
                                out=raw[:, slot0:slot0 + TPS]
                                    .unsqueeze(-1).broadcast_to((128, TPS, H)),
                                in0=strip3[:, :, 0:H],
                                in1=cp3[:, :, H:2 * H],
                                s0=-1.0e30,
                            )
                    # column maxes: partition max over the central WP columns
                    if fused and _EN_POOL:
                        cbase = (ub * NS + s) * TPS * WPAD
                        nc.gpsimd.partition_all_reduce(
                            colp[:, cbase:cbase + TPS * WPAD]
                                .rearrange("p (s w) -> p s w", w=WPAD)[:, :, 0:WP],
                            cp3[:, :, OFF:OFF + WP],
                            channels=128,
                            reduce_op=bass_isa.ReduceOp.max,
                        )
            nc.sync.dma_start(out=outt[:, :], in_=raw[:, :])
            if NPOOL:
                nc.sync.dma_start(
                    out=colo[:, :].rearrange("p (t w) -> p t w", w=WP),
                    in_=colp[0:1, :]
                        .rearrange("p (t w) -> p t w", w=WPAD)[:, :, 0:WP])
    nc.compile()
    _CACHE[key] = nc
    return nc


# -------------------------------------------------------------------- entry --

def _unit_xy(pred, target, b, orient):
    if orient == 0:
        return pred[b], target[b]
    return target[b], pred[b]


def _prepare_inputs(pred, target):
    units = _units()
    NU = len(units)
    ltabs = np.zeros((NCORES, NU, NGROUPS, K, GT * 128), BF)
    rtabs = np.zeros((NCORES, NU, NGROUPS, K, GT * W), BF)
    for core in range(NCORES):
        for ui, (ub, orient) in enumerate(units):
            b = core * BPC + ub
            X, Y = _unit_xy(pred, target, b, orient)
            Xs = X[_sort_perm(X)]
            Ys = Y[_sort_perm(Y)]
            lt, rt = _build_tabs(Xs, Ys)
            for i in range(NT):
                g, tg = i % NGROUPS, i // NGROUPS
                ltabs[core, ui, g, :, 128 * tg:128 * (tg + 1)] = \
                    lt[:, 128 * i:128 * (i + 1)]
                lo = _window_lo(i)
                rtabs[core, ui, g, :, W * tg:W * tg + W] = rt[:, lo:lo + W]
    return ltabs, rtabs


def _row_side(raw_slice, Xs, Ys):
    """Banded row-mins + z-gap certificate + exact host fallback.
    raw_slice: (128, NT) device row-maxes of -d^2.  Returns (rowmin, n_fb)."""
    kx = Xs[:, 2]
    ky = Ys[:, 2]
    rowmin = -raw_slice.T.reshape(-1).astype(np.float64)   # n = 128*i + p
    g = np.full(N, np.inf)
    for i in range(NT):
        rows = slice(128 * i, 128 * i + 128)
        lo = _window_lo(i)
        glo = np.maximum(kx[rows] - ky[lo - 1], 0) if lo > 0 else np.inf
        ghi = (np.maximum(ky[lo + W] - kx[rows], 0)
               if lo + W < M else np.inf)
        g[rows] = np.minimum(glo, ghi)
    bad = rowmin > g * g - CERT_SLACK
    if bad.any():
        d = ((Xs[bad, None, :] - Ys[None, :, :]) ** 2).sum(-1)
        rowmin[bad] = d.min(axis=1)
    return rowmin, int(bad.sum())


def _col_side(colo_slice, Xs, Ys):
    """Pool-path banded column-mins + certificate + fallback.
    colo_slice: flat (NT*WP,) device column-maxes of -d^2."""
    kx = Xs[:, 2]
    ky = Ys[:, 2]
    colmin = np.full(M, np.inf)
    cov_lo = np.full(M, NT, dtype=np.int64)
    cov_hi = np.full(M, -1, dtype=np.int64)
    for i in range(NT):
        s0 = _window_lo(i) + OFF
        vals = -colo_slice[i * WP:(i + 1) * WP]
        seg = slice(s0, s0 + WP)
        colmin[seg] = np.minimum(colmin[seg], vals)
        cov_lo[seg] = np.minimum(cov_lo[seg], i)
        cov_hi[seg] = np.maximum(cov_hi[seg], i)
    covered = cov_hi >= cov_lo
    lo_idx = cov_lo * 128 - 1
    gy_lo = np.where(lo_idx >= 0,
                     np.maximum(ky - kx[np.clip(lo_idx, 0, N - 1)], 0),
                     np.inf)
    hi_idx = (cov_hi + 1) * 128
    gy_hi = np.where(hi_idx < N,
                     np.maximum(kx[np.clip(hi_idx, 0, N - 1)] - ky, 0),
                     np.inf)
    gy = np.minimum(gy_lo, gy_hi)
    bad = (~covered) | (colmin > gy * gy - CERT_SLACK)
    if bad.any():
        d = ((Xs[None, :, :] - Ys[bad][:, None, :]) ** 2).sum(-1)
        colmin[bad] = d.min(axis=1)
    return colmin, int(bad.sum())


def _postprocess(results, pred, target):
    units = _units()
    losses = np.zeros(B)
    n_fb = 0
    for core in range(NCORES):
        raw = np.asarray(results[core]["out"])    # (128, NU*NT)
        colo = (np.asarray(results[core]["colo"]).reshape(-1)
                if "colo" in results[core] else None)
        sorted_xy = {}
        for ub in range(BPC):
            b = core * BPC + ub
            X, Y = pred[b], target[b]
            sorted_xy[ub] = (X[_sort_perm(X)].astype(np.float64),
                             Y[_sort_perm(Y)].astype(np.float64))
        for ui, (ub, orient) in enumerate(units):
            b = core * BPC + ub
            Xs, Ys = sorted_xy[ub]
            A, Bs = (Xs, Ys) if orient == 0 else (Ys, Xs)
            rowmin, fb = _row_side(raw[:, ui * NT:(ui + 1) * NT], A, Bs)
            n_fb += fb
            losses[b] += rowmin.mean()
        for ub in range(POOL_BATCHES):
            b = core * BPC + ub
            Xs, Ys = sorted_xy[ub]
            colmin, fb = _col_side(
                colo[ub * NT * WP:(ub + 1) * NT * WP], Xs, Ys)
            n_fb += fb
            losses[b] += colmin.mean()
    _CACHE["n_fallback"] = n_fb
    return np.float32(losses.mean())


def _run(pred, target, trace=False):
    from concourse.bass_utils import run_bass_kernel_spmd

    pred = np.asarray(pred, dtype=np.float32)
    target = np.asarray(target, dtype=np.float32)
    assert pred.shape == (B, N, C) and target.shape == (B, M, C)
    ltabs, rtabs = _prepare_inputs(pred, target)
    nc = _build_nc()
    in_maps = [{"ltab": ltabs[c], "rtab": rtabs[c]} for c in range(NCORES)]
    try:
        res = run_bass_kernel_spmd(nc, in_maps, core_ids=list(range(NCORES)),
                                   trace=trace)
    except Exception:
        res = run_bass_kernel_spmd(nc, in_maps, core_ids=list(range(NCORES)),
                                   trace=trace)
    return _postprocess(res.results, pred, target), res


def kernel(pred, target):
    loss, _ = _run(pred, target, trace=False)
    return loss
